# revision 15
# baseline (speedup 1.0000x reference)
"""DiffPool (nn_DiffPool_4715874091424) Trainium2 Bass kernel.

Math (reference is jax, B=32, C=CR=32, N=L=160, GDEP=2, ALPHA=0.05):
  A  = rownorm(a + I), A' = rownorm(a.T + I)
  mixprop folding:  embed = E0 x + E1 (M1 x) + E2 (M2 x) + 2 be
                    pool  = P0 x + P1 (M1 x) + P2 (M2 x) + 2 bp
  with M1 = A + A', M2 = A^2 + A'^2 (hop matrices), E*/P* folded 32x32
  channel-mix mats (host precompute).
  s = softmax_v(pool);  x_new[c] = s[c]^T @ embed[c];
  a_new[c] = (s[c] @ a) @ s[c].

Device pipeline per batch element b (8 cores, data-parallel over B, 4 b/core):
  1. x node-major streamed in chunks;  y12 = [M1|M2]^T.T @ x  (node matmuls)
  2. per (v,l)-segment: assemble hcat [96, seg] = [x_chan; y1_chan; y2_chan]
     (y rows via SBUF->SBUF strided DMA = the layout transpose), channel-mix
     matmul (Wcat [96,64]) + bias -> mixout -> DRAM scratch (chan-major)
  3. per c-group: reload pool/embed node-major from scratch, expP=exp(pool);
     x_new-MM with ones-column rhs yields softmax denom D as an extra output
     column; Dinv=1/D; PE-transpose expP -> s^T (Dinv-scaled on evict),
     transpose back -> s; tT = a^T s^T (const stationary); a_new = tT^T s.
"""

import sys

import numpy as np

if "/opt/trn_rl_repo" not in sys.path:
    sys.path.insert(0, "/opt/trn_rl_repo")

import concourse.bass as bass
import concourse.bacc as bacc
import concourse.mybir as mybir
import concourse.tile as tile
from concourse.bass_utils import run_bass_kernel_spmd
from concourse.masks import make_identity

F32 = mybir.dt.float32
AF = mybir.ActivationFunctionType

B, C, N, L = 32, 32, 160, 160
NCORES = 8
BPC = B // NCORES  # 4 batch elements per core
ALPHA, BETA = 0.05, 0.95
CL = C * L  # 5120
NSEG = 8
VQ = N // NSEG  # 20 node rows per (v,l) segment
QF = VQ * L  # 3200 free elements per segment
G = 4  # channels per phase-2 group
VT = [(0, 128), (128, 32)]  # partition tiles of the 160 node/cluster dim


def build_nc():
    nc = bacc.Bacc("TRN2", target_bir_lowering=False, debug=False, num_devices=NCORES)
    xs = nc.declare_dram_parameter("xs", [BPC, C, N, L], F32, isOutput=False)
    mt = nc.declare_dram_parameter("mt", [N, 2 * N], F32, isOutput=False)
    wcat = nc.declare_dram_parameter("wcat", [3 * C, 2 * C], F32, isOutput=False)
    b2 = nc.declare_dram_parameter("b2", [2 * C, 1], F32, isOutput=False)
    am = nc.declare_dram_parameter("am", [N, N], F32, isOutput=False)
    xn_out = nc.declare_dram_parameter("xn", [BPC, C, L, L], F32, isOutput=True)
    an_out = nc.declare_dram_parameter("an", [BPC, C, N, L], F32, isOutput=True)
    # scratch: chan-major mix output per b: [64, v, l] (embed rows 0:32, pool 32:64)
    mo = nc.dram_tensor("mo", [BPC, 2 * C, N, L], F32)
    # scratch: chan-major y1/y2 per b: [C, vstack 0:160=y1 160:320=y2, L]
    ys = nc.dram_tensor("ys", [BPC, C, 2 * N, L], F32)

    with tile.TileContext(nc) as tc:
        with (
            tc.tile_pool(name="consts", bufs=1) as pc,
            tc.tile_pool(name="work", bufs=1) as pw,
            tc.tile_pool(name="psum", bufs=1, space="PSUM") as pp,
        ):
            # ---- constants ----
            mt0 = pc.tile([128, 2 * N], F32)
            mt1 = pc.tile([32, 2 * N], F32)
            nc.sync.dma_start(mt0[:], mt[0:128, :])
            nc.sync.dma_start(mt1[:], mt[128:160, :])
            wc = pc.tile([3 * C, 2 * C], F32)
            nc.sync.dma_start(wc[:], wcat[:])
            b2c = pc.tile([2 * C, 1], F32)
            nc.sync.dma_start(b2c[:], b2[:])
            am0 = pc.tile([128, N], F32)
            am1 = pc.tile([32, N], F32)
            nc.sync.dma_start(am0[:], am[0:128, :])
            nc.sync.dma_start(am1[:], am[128:160, :])
            ident = pc.tile([128, 128], F32)
            make_identity(nc, ident[:])

            for b in range(BPC):
                _phase1(nc, pw, pp, xs, mo, ys, b, mt0, mt1, wc, b2c)
                _phase2(nc, pw, pp, mo, xn_out, an_out, b, am0, am1, ident)

    return nc


def _ps(pp, shape):
    return pp.tile(shape, F32, tag="ps", name="ps", bufs=6)


def _phase1(nc, pw, pp, xs, mo, ys, b, mt0, mt1, wc, b2c):
    """y12 node matmuls + chan-mix -> mo[b] (chan-major)."""
    xnode = xs[b].rearrange("c w l -> w c l")  # [160, 32, 160]
    MTILES = [(0, 128), (128, 128), (256, 64)]
    MTs = [mt0, mt1]

    # y-stack rows: 0:160 = y1 = M1 x, 160:320 = y2 = M2 x
    Y = [
        pw.tile([128, CL], F32, tag="y0", name="y0"),
        pw.tile([128, CL], F32, tag="y1", name="y1"),
        pw.tile([64, CL], F32, tag="y2", name="y2"),
    ]
    # x node-major streamed in chunks of 2 channels (N=320 columns)
    for ch in range(16):
        xc0 = pw.tile([128, 320], F32, tag="xc0", name="xc0", bufs=3)
        xc1 = pw.tile([32, 320], F32, tag="xc1", name="xc1", bufs=3)
        nc.sync.dma_start(
            xc0[:].rearrange("p (c l) -> p c l", c=2),
            xnode[0:128, 2 * ch : 2 * ch + 2, :],
        )
        nc.sync.dma_start(
            xc1[:].rearrange("p (c l) -> p c l", c=2),
            xnode[128:160, 2 * ch : 2 * ch + 2, :],
        )
        xcs = [xc0, xc1]
        for mi, (m0, msz) in enumerate(MTILES):
            ps = _ps(pp, [128, 320])
            for kt in range(2):
                nc.tensor.matmul(
                    ps[:msz, :],
                    MTs[kt][:, m0 : m0 + msz],
                    xcs[kt][:],
                    start=(kt == 0),
                    stop=(kt == 1),
                )
            nc.vector.tensor_copy(Y[mi][:msz, ch * 320 : (ch + 1) * 320], ps[:msz, :])

    # node->chan layout transpose of y12 via DRAM roundtrip (640B runs each way)
    for mi, (m0, msz) in enumerate(MTILES):
        nc.sync.dma_start(
            ys[b][:, m0 : m0 + msz, :].rearrange("c v l -> v c l"),
            Y[mi][:].rearrange("v (c l) -> v c l", c=C),
        )

    # per (v,l)-segment: hcat = [x_chan; y1_chan; y2_chan] [96, QF] -> mix
    for q in range(NSEG):
        v0 = q * VQ
        hq = pw.tile([3 * C, QF], F32, tag="hcat", name="hcat", bufs=2)
        # x rows (chan-major from DRAM)
        nc.sync.dma_start(
            hq[0:C, :].rearrange("c (v l) -> c v l", v=VQ),
            xs[b][:, v0 : v0 + VQ, :],
        )
        # y rows (chan-major from ys scratch)
        for blk, base in ((1, 0), (2, N)):  # hcat block 1 => y1, 2 => y2
            nc.sync.dma_start(
                hq[blk * C : (blk + 1) * C, :].rearrange("c (v l) -> c v l", v=VQ),
                ys[b][:, base + v0 : base + v0 + VQ, :],
            )
        # mix: out[o, pos] = sum_c' wc[c', o] * hq[c', pos], + bias
        moq = pw.tile([2 * C, QF], F32, tag="moq", name="moq", bufs=2)
        for ch in range(8):  # 8 chunks of 400
            ps = _ps(pp, [64, 400])
            nc.tensor.matmul(
                ps[:, :], wc[:], hq[:, ch * 400 : (ch + 1) * 400],
                start=True, stop=True,
            )
            nc.vector.tensor_scalar_add(
                moq[:, ch * 400 : (ch + 1) * 400], ps[:, :], b2c[:]
            )
        nc.sync.dma_start(
            mo[b][:, v0 : v0 + VQ, :],
            moq[:].rearrange("o (v l) -> o v l", v=VQ),
        )


def _phase2(nc, pw, pp, mo, xn_out, an_out, b, am0, am1, ident):
    """softmax + x_new + a_new per c-group of G."""
    ams = [am0, am1]
    for g in range(C // G):
        c0 = g * G
        # pool (rows 32:64 of mo) / embed (rows 0:32) node-major [v, (c,l)]
        pgs, egs = [], []
        for i, (v0, sz) in enumerate(VT):
            pg = pw.tile([sz, G * L], F32, tag=f"pg{i}", name=f"pg{i}", bufs=2)
            eg = pw.tile([sz, G * (L + 1)], F32, tag=f"eg{i}", name=f"eg{i}", bufs=2)
            nc.sync.dma_start(
                pg[:].rearrange("v (c l) -> v c l", c=G),
                mo[b][C + c0 : C + c0 + G, v0 : v0 + sz, :].rearrange("c v l -> v c l"),
            )
            nc.sync.dma_start(
                eg[:].rearrange("v (c l) -> v c l", c=G)[:, :, 0:L],
                mo[b][c0 : c0 + G, v0 : v0 + sz, :].rearrange("c v l -> v c l"),
            )
            nc.gpsimd.memset(
                eg[:].rearrange("v (c l) -> v c l", c=G)[:, :, L : L + 1], 1.0
            )
            pgs.append(pg)
            egs.append(eg)
        # expP = exp(pool)
        xps = [
            pw.tile([sz, G * L], F32, tag=f"xp{i}", name=f"xp{i}")
            for i, (_, sz) in enumerate(VT)
        ]
        for i in range(2):
            nc.scalar.activation(xps[i][:], pgs[i][:], AF.Exp)
        dvs = [
            pw.tile([sz, G], F32, tag=f"dv{i}", name=f"dv{i}")
            for i, (_, sz) in enumerate(VT)
        ]
        slg = [
            pw.tile([sz, G * N], F32, tag=f"sl{i}", name=f"sl{i}")
            for i, (_, sz) in enumerate(VT)
        ]
        sng = [
            pw.tile([sz, G * L], F32, tag=f"sn{i}", name=f"sn{i}")
            for i, (_, sz) in enumerate(VT)
        ]
        xgs = [
            pw.tile([sz, G * L], F32, tag=f"xg{i}", name=f"xg{i}", bufs=2)
            for i, (_, sz) in enumerate(VT)
        ]

        for ci in range(G):
            # ---- x_new: raw = expP[c]^T @ [e[c] | 1];  D = last col ----
            for mi, (m0, msz) in enumerate(VT):  # l1 tiles
                ps = _ps(pp, [128, L + 1])
                for kt, (k0, ksz) in enumerate(VT):  # v tiles
                    nc.tensor.matmul(
                        ps[:msz, :],
                        xps[kt][:, ci * L + m0 : ci * L + m0 + msz],
                        egs[kt][:, ci * (L + 1) : (ci + 1) * (L + 1)],
                        start=(kt == 0),
                        stop=(kt == 1),
                    )
                nc.vector.reciprocal(dvs[mi][:msz, ci : ci + 1], ps[:msz, L : L + 1])
                nc.scalar.activation(
                    xgs[mi][:msz, ci * L : (ci + 1) * L],
                    ps[:msz, 0:L],
                    AF.Copy,
                    scale=dvs[mi][:msz, ci : ci + 1],
                )
            # ---- T1: s^T[c] = transpose(expP[c]) * Dinv  (l-major) ----
            for kt, (k0, ksz) in enumerate(VT):  # source v tile
                for lt, (l0, lsz) in enumerate(VT):  # source l cols
                    ps = _ps(pp, [128, 128])
                    nc.tensor.transpose(
                        ps[:lsz, :ksz],
                        xps[kt][:, ci * L + l0 : ci * L + l0 + lsz],
                        ident[:ksz, :ksz],
                    )
                    nc.scalar.activation(
                        slg[lt][:lsz, ci * N + k0 : ci * N + k0 + ksz],
                        ps[:lsz, :ksz],
                        AF.Copy,
                        scale=dvs[lt][:lsz, ci : ci + 1],
                    )
            # ---- T2: s[c] = transpose(s^T[c])  (node-major) ----
            for kt, (k0, ksz) in enumerate(VT):  # source l tile
                for vt, (v0, vsz) in enumerate(VT):  # source v cols
                    ps = _ps(pp, [128, 128])
                    nc.tensor.transpose(
                        ps[:vsz, :ksz],
                        slg[kt][:, ci * N + v0 : ci * N + v0 + vsz],
                        ident[:ksz, :ksz],
                    )
                    nc.vector.tensor_copy(
                        sng[vt][:vsz, ci * L + k0 : ci * L + k0 + ksz],
                        ps[:vsz, :ksz],
                    )
        # ---- tT = a^T s^T : tT[j, (c,v)] = sum_k a[k,j] s_l[k, (c,v)] ----
        ttg = [
            pw.tile([sz, G * N], F32, tag=f"tt{i}", name=f"tt{i}")
            for i, (_, sz) in enumerate(VT)
        ]
        NCH = G * N // 320  # chunks of 320
        for mi, (m0, msz) in enumerate(VT):  # j tiles
            for ch in range(NCH):
                ps = _ps(pp, [128, 320])
                for kt in range(2):
                    nc.tensor.matmul(
                        ps[:msz, :],
                        ams[kt][:, m0 : m0 + msz],
                        slg[kt][:, ch * 320 : (ch + 1) * 320],
                        start=(kt == 0),
                        stop=(kt == 1),
                    )
                nc.vector.tensor_copy(
                    ttg[mi][:msz, ch * 320 : (ch + 1) * 320], ps[:msz, :]
                )
        # ---- a_new[c] = tT[c]^T @ s[c] ----
        ang = [
            pw.tile([sz, G * L], F32, tag=f"ag{i}", name=f"ag{i}", bufs=2)
            for i, (_, sz) in enumerate(VT)
        ]
        for ci in range(G):
            for mi, (m0, msz) in enumerate(VT):  # v tiles (output partition)
                ps = _ps(pp, [128, L])
                for kt, (k0, ksz) in enumerate(VT):  # j tiles
                    nc.tensor.matmul(
                        ps[:msz, :],
                        ttg[kt][:, ci * N + m0 : ci * N + m0 + msz],
                        sng[kt][:, ci * L : (ci + 1) * L],
                        start=(kt == 0),
                        stop=(kt == 1),
                    )
                nc.vector.tensor_copy(ang[mi][:msz, ci * L : (ci + 1) * L], ps[:msz, :])
        # ---- outputs ----
        for i, (v0, sz) in enumerate(VT):
            nc.sync.dma_start(
                xn_out[b][c0 : c0 + G, v0 : v0 + sz, :].rearrange("c p q -> p c q"),
                xgs[i][:sz].rearrange("p (c q) -> p c q", c=G),
            )
            nc.sync.dma_start(
                an_out[b][c0 : c0 + G, v0 : v0 + sz, :].rearrange("c p q -> p c q"),
                ang[i][:sz].rearrange("p (c q) -> p c q", c=G),
            )


def _host_prep(x, a, We, be, Wp, bp):
    a = np.asarray(a, np.float64)
    I = np.eye(N, dtype=np.float64)
    A1 = (a + I) / (a + I).sum(1, keepdims=True)
    A2 = (a.T + I) / (a.T + I).sum(1, keepdims=True)
    M1 = A1 + A2
    M2 = A1 @ A1 + A2 @ A2
    MT = np.concatenate([M1.T, M2.T], axis=1).astype(np.float32)  # [N, 2N]

    def fold(W):
        W = np.asarray(W, np.float64)
        W0, W1, W2 = W[:, :C], W[:, C : 2 * C], W[:, 2 * C :]
        F0 = 2.0 * (W0 + ALPHA * W1 + ALPHA * W2)
        F1 = BETA * W1 + ALPHA * BETA * W2
        F2 = BETA * BETA * W2
        return F0, F1, F2

    E0, E1, E2 = fold(We)
    P0, P1, P2 = fold(Wp)
    # lhsT[c', o]: rows = [x-block; y1-block; y2-block], cols = [e outs | pool outs]
    Wcat = np.block([[E0.T, P0.T], [E1.T, P1.T], [E2.T, P2.T]]).astype(np.float32)
    b2 = np.concatenate([2.0 * np.asarray(be), 2.0 * np.asarray(bp)]).astype(
        np.float32
    )[:, None]
    return MT, Wcat, b2, np.asarray(a, np.float32)


def _install_ntff_shim():
    """Provide antenv.axon_hooks (missing in this image) so
    run_bass_kernel_spmd(trace=True) can drive NTFF profiling via the
    axon PJRT .so. No-op if anything is unavailable."""
    import contextlib
    import ctypes
    import types

    try:
        import antenv  # noqa: F401

        try:
            from antenv.axon_hooks import get_axon_ntff_profile_hook  # noqa: F401

            return
        except ImportError:
            pass
        lib = ctypes.CDLL("/opt/axon/libaxon_pjrt.so")
        if not hasattr(lib, "axon_start_nrt_profile"):
            return
        lib.axon_start_nrt_profile.argtypes = [
            ctypes.POINTER(ctypes.c_int64),
            ctypes.c_size_t,
        ]
        lib.axon_start_nrt_profile.restype = ctypes.c_int64
        lib.axon_stop_nrt_profile.argtypes = [ctypes.c_char_p]
        lib.axon_stop_nrt_profile.restype = ctypes.c_int64

        @contextlib.contextmanager
        def _hook(output_dir, device_ids):
            import jax

            jax.devices()
            if device_ids:
                ids = (ctypes.c_int64 * len(device_ids))(*device_ids)
                rc = lib.axon_start_nrt_profile(ids, len(device_ids))
            else:
                rc = lib.axon_start_nrt_profile(None, 0)
            if rc != 0:
                raise RuntimeError(f"axon_start_nrt_profile rc={rc}")
            try:
                yield
            finally:
                n = lib.axon_stop_nrt_profile(str(output_dir).encode())
                print(f"ntff profile: {n} file(s) -> {output_dir}", file=sys.stderr)

        holder = {"h": _hook}
        mod = types.ModuleType("antenv.axon_hooks")
        mod.get_axon_ntff_profile_hook = lambda: holder["h"]
        mod.set_axon_ntff_profile_hook = lambda h: holder.__setitem__("h", h)
        sys.modules["antenv.axon_hooks"] = mod
        antenv.axon_hooks = mod
    except Exception as e:  # pragma: no cover
        print(f"ntff shim unavailable: {e}", file=sys.stderr)


_NC_CACHE = {}


def _get_nc():
    if "nc" not in _NC_CACHE:
        nc = build_nc()
        nc.compile()  # bacc lowering: wait-splitting, register allocation, ...
        _NC_CACHE["nc"] = nc
    return _NC_CACHE["nc"]


def run_spmd(x, a, We, be, Wp, bp, trace=False):
    if trace:
        _install_ntff_shim()
    x = np.ascontiguousarray(np.asarray(x, np.float32))
    MT, Wcat, b2, a32 = _host_prep(x, a, We, be, Wp, bp)
    nc = _get_nc()
    in_maps = [
        {
            "xs": x[i * BPC : (i + 1) * BPC],
            "mt": MT,
            "wcat": Wcat,
            "b2": b2,
            "am": a32,
        }
        for i in range(NCORES)
    ]
    res = run_bass_kernel_spmd(nc, in_maps, list(range(NCORES)), trace=trace)
    xn = np.concatenate([res.results[i]["xn"] for i in range(NCORES)], axis=0)
    an = np.concatenate([res.results[i]["an"] for i in range(NCORES)], axis=0)
    return (xn, an), res


def kernel(x, a, We, be, Wp, bp):
    (xn, an), _ = run_spmd(x, a, We, be, Wp, bp, trace=False)
    return (xn, an)


# revision 18
# speedup vs baseline: 1.4781x; 1.4781x over previous
"""DiffPool (nn_DiffPool_4715874091424) Trainium2 Bass kernel.

Math (reference is jax, B=32, C=CR=32, N=L=160, GDEP=2, ALPHA=0.05):
  A  = rownorm(a + I), A' = rownorm(a.T + I)
  mixprop folding:  embed = E0 x + E1 (M1 x) + E2 (M2 x) + 2 be
                    pool  = P0 x + P1 (M1 x) + P2 (M2 x) + 2 bp
  with M1 = A + A', M2 = A^2 + A'^2 (hop matrices), E*/P* folded 32x32
  channel-mix mats (host precompute).
  s = softmax_v(pool);  x_new[c] = s[c]^T @ embed[c];
  a_new[c] = (s[c] @ a) @ s[c].

Device pipeline per batch element b (8 cores, data-parallel over B, 4 b/core):
  1. x node-major streamed in chunks;  y12 = [M1|M2]^T.T @ x  (node matmuls)
  2. per (v,l)-segment: assemble hcat [96, seg] = [x_chan; y1_chan; y2_chan]
     (y rows via SBUF->SBUF strided DMA = the layout transpose), channel-mix
     matmul (Wcat [96,64]) + bias -> mixout -> DRAM scratch (chan-major)
  3. per c-group: reload pool/embed node-major from scratch, expP=exp(pool);
     x_new-MM with ones-column rhs yields softmax denom D as an extra output
     column; Dinv=1/D; PE-transpose expP -> s^T (Dinv-scaled on evict),
     transpose back -> s; tT = a^T s^T (const stationary); a_new = tT^T s.
"""

import sys

import numpy as np

if "/opt/trn_rl_repo" not in sys.path:
    sys.path.insert(0, "/opt/trn_rl_repo")

import concourse.bass as bass
import concourse.bacc as bacc
import concourse.mybir as mybir
import concourse.tile as tile
from concourse.bass_utils import run_bass_kernel_spmd
from concourse.masks import make_identity

F32 = mybir.dt.float32
AF = mybir.ActivationFunctionType

B, C, N, L = 32, 32, 160, 160
NCORES = 8
BPC = B // NCORES  # 4 batch elements per core
ALPHA, BETA = 0.05, 0.95
CL = C * L  # 5120
NSEG = 8
VQ = N // NSEG  # 20 node rows per (v,l) segment
QF = VQ * L  # 3200 free elements per segment
G = 4  # channels per phase-2 group
VT = [(0, 128), (128, 32)]  # partition tiles of the 160 node/cluster dim


def build_nc():
    nc = bacc.Bacc("TRN2", target_bir_lowering=False, debug=False, num_devices=NCORES)
    xs = nc.declare_dram_parameter("xs", [BPC, C, N, L], F32, isOutput=False)
    mt = nc.declare_dram_parameter("mt", [N, 2 * N], F32, isOutput=False)
    wcat = nc.declare_dram_parameter("wcat", [3 * C, 2 * C], F32, isOutput=False)
    b2 = nc.declare_dram_parameter("b2", [2 * C, 1], F32, isOutput=False)
    am = nc.declare_dram_parameter("am", [N, N], F32, isOutput=False)
    xn_out = nc.declare_dram_parameter("xn", [BPC, C, L, L], F32, isOutput=True)
    an_out = nc.declare_dram_parameter("an", [BPC, C, N, L], F32, isOutput=True)
    # scratch: chan-major mix output per b: [64, v, l] (embed rows 0:32, pool 32:64)
    mo = nc.dram_tensor("mo", [BPC, 2 * C, N, L], F32)
    # scratch: chan-major y1/y2 per b: [C, vstack 0:160=y1 160:320=y2, L]
    ys = nc.dram_tensor("ys", [BPC, C, 2 * N, L], F32)

    with tile.TileContext(nc) as tc:
        with (
            tc.tile_pool(name="consts", bufs=1) as pc,
            tc.tile_pool(name="work", bufs=1) as pw,
            tc.tile_pool(name="psum", bufs=1, space="PSUM") as pp,
        ):
            # ---- constants ----
            mt0 = pc.tile([128, 2 * N], F32)
            mt1 = pc.tile([32, 2 * N], F32)
            nc.sync.dma_start(mt0[:], mt[0:128, :])
            nc.sync.dma_start(mt1[:], mt[128:160, :])
            wc = pc.tile([3 * C, 2 * C], F32)
            nc.sync.dma_start(wc[:], wcat[:])
            b2c = pc.tile([2 * C, 1], F32)
            nc.sync.dma_start(b2c[:], b2[:])
            am0 = pc.tile([128, N], F32)
            am1 = pc.tile([32, N], F32)
            nc.sync.dma_start(am0[:], am[0:128, :])
            nc.sync.dma_start(am1[:], am[128:160, :])
            ident = pc.tile([128, 128], F32)
            make_identity(nc, ident[:])

            for b in range(BPC):
                _phase1(nc, pw, pp, xs, mo, ys, b, mt0, mt1, wc, b2c)
                _phase2(nc, pw, pp, mo, xn_out, an_out, b, am0, am1, ident)

    return nc


def _psA(pp, shape):
    return pp.tile(shape, F32, tag="psA", name="psA", bufs=4)


def _psB(pp, shape):
    return pp.tile(shape, F32, tag="psB", name="psB", bufs=4)


def _phase1(nc, pw, pp, xs, mo, ys, b, mt0, mt1, wc, b2c):
    """y12 node matmuls + chan-mix -> mo[b] (chan-major)."""
    xnode = xs[b].rearrange("c w l -> w c l")  # [160, 32, 160]
    MTILES = [(0, 128), (128, 128), (256, 64)]
    MTs = [mt0, mt1]

    # y-stack rows: 0:160 = y1 = M1 x, 160:320 = y2 = M2 x
    Y = [
        pw.tile([128, CL], F32, tag="y0", name="y0"),
        pw.tile([128, CL], F32, tag="y1", name="y1"),
        pw.tile([64, CL], F32, tag="y2", name="y2"),
    ]
    # x node-major streamed in groups of 8 channels ([*, 1280] tiles);
    # within a group: 4 sub-chunks of N=320 share each stationary (LDW amortized)
    for grp in range(4):
        xc0 = pw.tile([128, 1280], F32, tag="xc0", name="xc0", bufs=2)
        xc1 = pw.tile([32, 1280], F32, tag="xc1", name="xc1", bufs=2)
        nc.sync.dma_start(
            xc0[:].rearrange("p (c l) -> p c l", c=8),
            xnode[0:128, 8 * grp : 8 * grp + 8, :],
        )
        nc.sync.dma_start(
            xc1[:].rearrange("p (c l) -> p c l", c=8),
            xnode[128:160, 8 * grp : 8 * grp + 8, :],
        )
        xcs = [xc0, xc1]
        for mi, (m0, msz) in enumerate(MTILES):
            pss = [_psA(pp, [128, 320]) for _ in range(4)]
            for kt in range(2):
                for sub in range(4):
                    nc.tensor.matmul(
                        pss[sub][:msz, :],
                        MTs[kt][:, m0 : m0 + msz],
                        xcs[kt][:, sub * 320 : (sub + 1) * 320],
                        start=(kt == 0),
                        stop=(kt == 1),
                    )
            for sub in range(4):
                nc.vector.tensor_copy(
                    Y[mi][:msz, (4 * grp + sub) * 320 : (4 * grp + sub + 1) * 320],
                    pss[sub][:msz, :],
                )

    # node->chan layout transpose of y12 via DRAM roundtrip (640B runs each way)
    for mi, (m0, msz) in enumerate(MTILES):
        nc.gpsimd.dma_start(
            ys[b][:, m0 : m0 + msz, :].rearrange("c v l -> v c l"),
            Y[mi][:].rearrange("v (c l) -> v c l", c=C),
        )

    # per (v,l)-segment: hcat = [x_chan; y1_chan; y2_chan] [96, QF] -> mix
    for q in range(NSEG):
        v0 = q * VQ
        hq = pw.tile([3 * C, QF], F32, tag="hcat", name="hcat", bufs=2)
        # x rows (chan-major from DRAM)
        nc.sync.dma_start(
            hq[0:C, :].rearrange("c (v l) -> c v l", v=VQ),
            xs[b][:, v0 : v0 + VQ, :],
        )
        # y rows (chan-major from ys scratch)
        for blk, base in ((1, 0), (2, N)):  # hcat block 1 => y1, 2 => y2
            nc.sync.dma_start(
                hq[blk * C : (blk + 1) * C, :].rearrange("c (v l) -> c v l", v=VQ),
                ys[b][:, base + v0 : base + v0 + VQ, :],
            )
        # mix: out[o, pos] = sum_c' wc[c', o] * hq[c', pos], + bias
        moq = pw.tile([2 * C, QF], F32, tag="moq", name="moq", bufs=2)
        for ch in range(8):  # 8 chunks of 400
            ps = _psB(pp, [64, 400])
            nc.tensor.matmul(
                ps[:, :], wc[:], hq[:, ch * 400 : (ch + 1) * 400],
                start=True, stop=True,
            )
            nc.vector.tensor_scalar_add(
                moq[:, ch * 400 : (ch + 1) * 400], ps[:, :], b2c[:]
            )
        nc.gpsimd.dma_start(
            mo[b][:, v0 : v0 + VQ, :],
            moq[:].rearrange("o (v l) -> o v l", v=VQ),
        )


def _phase2(nc, pw, pp, mo, xn_out, an_out, b, am0, am1, ident):
    """softmax + x_new + a_new per c-group of G."""
    ams = [am0, am1]
    for g in range(C // G):
        c0 = g * G
        # pool (rows 32:64 of mo) / embed (rows 0:32) node-major [v, (c,l)]
        pgs, egs = [], []
        for i, (v0, sz) in enumerate(VT):
            pg = pw.tile([sz, G * L], F32, tag=f"pg{i}", name=f"pg{i}", bufs=2)
            eg = pw.tile([sz, G * (L + 1)], F32, tag=f"eg{i}", name=f"eg{i}", bufs=2)
            nc.sync.dma_start(
                pg[:].rearrange("v (c l) -> v c l", c=G),
                mo[b][C + c0 : C + c0 + G, v0 : v0 + sz, :].rearrange("c v l -> v c l"),
            )
            nc.sync.dma_start(
                eg[:].rearrange("v (c l) -> v c l", c=G)[:, :, 0:L],
                mo[b][c0 : c0 + G, v0 : v0 + sz, :].rearrange("c v l -> v c l"),
            )
            nc.vector.memset(
                eg[:].rearrange("v (c l) -> v c l", c=G)[:, :, L : L + 1], 1.0
            )
            pgs.append(pg)
            egs.append(eg)
        # expP = exp(pool)
        xps = [
            pw.tile([sz, G * L], F32, tag=f"xp{i}", name=f"xp{i}")
            for i, (_, sz) in enumerate(VT)
        ]
        for i in range(2):
            nc.scalar.activation(xps[i][:], pgs[i][:], AF.Exp)
        dvs = [
            pw.tile([sz, G], F32, tag=f"dv{i}", name=f"dv{i}")
            for i, (_, sz) in enumerate(VT)
        ]
        slg = [
            pw.tile([sz, G * N], F32, tag=f"sl{i}", name=f"sl{i}")
            for i, (_, sz) in enumerate(VT)
        ]
        sng = [
            pw.tile([sz, G * L], F32, tag=f"sn{i}", name=f"sn{i}")
            for i, (_, sz) in enumerate(VT)
        ]
        xgs = [
            pw.tile([sz, G * L], F32, tag=f"xg{i}", name=f"xg{i}", bufs=2)
            for i, (_, sz) in enumerate(VT)
        ]

        for ci in range(G):
            # ---- x_new: raw = expP[c]^T @ [e[c] | 1];  D = last col ----
            for mi, (m0, msz) in enumerate(VT):  # l1 tiles
                ps = _psA(pp, [128, L + 1])
                for kt, (k0, ksz) in enumerate(VT):  # v tiles
                    nc.tensor.matmul(
                        ps[:msz, :],
                        xps[kt][:, ci * L + m0 : ci * L + m0 + msz],
                        egs[kt][:, ci * (L + 1) : (ci + 1) * (L + 1)],
                        start=(kt == 0),
                        stop=(kt == 1),
                    )
                nc.vector.reciprocal(dvs[mi][:msz, ci : ci + 1], ps[:msz, L : L + 1])
                nc.scalar.activation(
                    xgs[mi][:msz, ci * L : (ci + 1) * L],
                    ps[:msz, 0:L],
                    AF.Copy,
                    scale=dvs[mi][:msz, ci : ci + 1],
                )
            # ---- T1: s^T[c] = transpose(expP[c]) * Dinv  (l-major) ----
            for kt, (k0, ksz) in enumerate(VT):  # source v tile
                for lt, (l0, lsz) in enumerate(VT):  # source l cols
                    ps = _psA(pp, [128, 128])
                    nc.tensor.transpose(
                        ps[:lsz, :ksz],
                        xps[kt][:, ci * L + l0 : ci * L + l0 + lsz],
                        ident[:ksz, :ksz],
                    )
                    nc.scalar.activation(
                        slg[lt][:lsz, ci * N + k0 : ci * N + k0 + ksz],
                        ps[:lsz, :ksz],
                        AF.Copy,
                        scale=dvs[lt][:lsz, ci : ci + 1],
                    )
            # ---- T2: s[c] = transpose(s^T[c])  (node-major) ----
            for kt, (k0, ksz) in enumerate(VT):  # source l tile
                for vt, (v0, vsz) in enumerate(VT):  # source v cols
                    ps = _psB(pp, [128, 128])
                    nc.tensor.transpose(
                        ps[:vsz, :ksz],
                        slg[kt][:, ci * N + v0 : ci * N + v0 + vsz],
                        ident[:ksz, :ksz],
                    )
                    nc.vector.tensor_copy(
                        sng[vt][:vsz, ci * L + k0 : ci * L + k0 + ksz],
                        ps[:vsz, :ksz],
                    )
        # ---- tT = a^T s^T : tT[j, (c,v)] = sum_k a[k,j] s_l[k, (c,v)] ----
        ttg = [
            pw.tile([sz, G * N], F32, tag=f"tt{i}", name=f"tt{i}")
            for i, (_, sz) in enumerate(VT)
        ]
        NCH = G * N // 320  # chunks of 320
        for mi, (m0, msz) in enumerate(VT):  # j tiles
            for ch in range(NCH):
                ps = _psB(pp, [128, 320])
                for kt in range(2):
                    nc.tensor.matmul(
                        ps[:msz, :],
                        ams[kt][:, m0 : m0 + msz],
                        slg[kt][:, ch * 320 : (ch + 1) * 320],
                        start=(kt == 0),
                        stop=(kt == 1),
                    )
                nc.vector.tensor_copy(
                    ttg[mi][:msz, ch * 320 : (ch + 1) * 320], ps[:msz, :]
                )
        # ---- a_new[c] = tT[c]^T @ s[c] ----
        ang = [
            pw.tile([sz, G * L], F32, tag=f"ag{i}", name=f"ag{i}", bufs=2)
            for i, (_, sz) in enumerate(VT)
        ]
        for ci in range(G):
            for mi, (m0, msz) in enumerate(VT):  # v tiles (output partition)
                ps = _psB(pp, [128, L])
                for kt, (k0, ksz) in enumerate(VT):  # j tiles
                    nc.tensor.matmul(
                        ps[:msz, :],
                        ttg[kt][:, ci * N + m0 : ci * N + m0 + msz],
                        sng[kt][:, ci * L : (ci + 1) * L],
                        start=(kt == 0),
                        stop=(kt == 1),
                    )
                nc.vector.tensor_copy(ang[mi][:msz, ci * L : (ci + 1) * L], ps[:msz, :])
        # ---- outputs ----
        for i, (v0, sz) in enumerate(VT):
            nc.gpsimd.dma_start(
                xn_out[b][c0 : c0 + G, v0 : v0 + sz, :].rearrange("c p q -> p c q"),
                xgs[i][:sz].rearrange("p (c q) -> p c q", c=G),
            )
            nc.gpsimd.dma_start(
                an_out[b][c0 : c0 + G, v0 : v0 + sz, :].rearrange("c p q -> p c q"),
                ang[i][:sz].rearrange("p (c q) -> p c q", c=G),
            )


def _host_prep(x, a, We, be, Wp, bp):
    a = np.asarray(a, np.float64)
    I = np.eye(N, dtype=np.float64)
    A1 = (a + I) / (a + I).sum(1, keepdims=True)
    A2 = (a.T + I) / (a.T + I).sum(1, keepdims=True)
    M1 = A1 + A2
    M2 = A1 @ A1 + A2 @ A2
    MT = np.concatenate([M1.T, M2.T], axis=1).astype(np.float32)  # [N, 2N]

    def fold(W):
        W = np.asarray(W, np.float64)
        W0, W1, W2 = W[:, :C], W[:, C : 2 * C], W[:, 2 * C :]
        F0 = 2.0 * (W0 + ALPHA * W1 + ALPHA * W2)
        F1 = BETA * W1 + ALPHA * BETA * W2
        F2 = BETA * BETA * W2
        return F0, F1, F2

    E0, E1, E2 = fold(We)
    P0, P1, P2 = fold(Wp)
    # lhsT[c', o]: rows = [x-block; y1-block; y2-block], cols = [e outs | pool outs]
    Wcat = np.block([[E0.T, P0.T], [E1.T, P1.T], [E2.T, P2.T]]).astype(np.float32)
    b2 = np.concatenate([2.0 * np.asarray(be), 2.0 * np.asarray(bp)]).astype(
        np.float32
    )[:, None]
    return MT, Wcat, b2, np.asarray(a, np.float32)


def _install_ntff_shim():
    """Provide antenv.axon_hooks (missing in this image) so
    run_bass_kernel_spmd(trace=True) can drive NTFF profiling via the
    axon PJRT .so. No-op if anything is unavailable."""
    import contextlib
    import ctypes
    import types

    try:
        import antenv  # noqa: F401

        try:
            from antenv.axon_hooks import get_axon_ntff_profile_hook  # noqa: F401

            return
        except ImportError:
            pass
        lib = ctypes.CDLL("/opt/axon/libaxon_pjrt.so")
        if not hasattr(lib, "axon_start_nrt_profile"):
            return
        lib.axon_start_nrt_profile.argtypes = [
            ctypes.POINTER(ctypes.c_int64),
            ctypes.c_size_t,
        ]
        lib.axon_start_nrt_profile.restype = ctypes.c_int64
        lib.axon_stop_nrt_profile.argtypes = [ctypes.c_char_p]
        lib.axon_stop_nrt_profile.restype = ctypes.c_int64

        @contextlib.contextmanager
        def _hook(output_dir, device_ids):
            import jax

            jax.devices()
            if device_ids:
                ids = (ctypes.c_int64 * len(device_ids))(*device_ids)
                rc = lib.axon_start_nrt_profile(ids, len(device_ids))
            else:
                rc = lib.axon_start_nrt_profile(None, 0)
            if rc != 0:
                raise RuntimeError(f"axon_start_nrt_profile rc={rc}")
            try:
                yield
            finally:
                n = lib.axon_stop_nrt_profile(str(output_dir).encode())
                print(f"ntff profile: {n} file(s) -> {output_dir}", file=sys.stderr)

        holder = {"h": _hook}
        mod = types.ModuleType("antenv.axon_hooks")
        mod.get_axon_ntff_profile_hook = lambda: holder["h"]
        mod.set_axon_ntff_profile_hook = lambda h: holder.__setitem__("h", h)
        sys.modules["antenv.axon_hooks"] = mod
        antenv.axon_hooks = mod
    except Exception as e:  # pragma: no cover
        print(f"ntff shim unavailable: {e}", file=sys.stderr)


_NC_CACHE = {}


def _get_nc():
    if "nc" not in _NC_CACHE:
        nc = build_nc()
        nc.compile()  # bacc lowering: wait-splitting, register allocation, ...
        _NC_CACHE["nc"] = nc
    return _NC_CACHE["nc"]


def run_spmd(x, a, We, be, Wp, bp, trace=False):
    if trace:
        _install_ntff_shim()
    x = np.ascontiguousarray(np.asarray(x, np.float32))
    MT, Wcat, b2, a32 = _host_prep(x, a, We, be, Wp, bp)
    nc = _get_nc()
    in_maps = [
        {
            "xs": x[i * BPC : (i + 1) * BPC],
            "mt": MT,
            "wcat": Wcat,
            "b2": b2,
            "am": a32,
        }
        for i in range(NCORES)
    ]
    res = run_bass_kernel_spmd(nc, in_maps, list(range(NCORES)), trace=trace)
    xn = np.concatenate([res.results[i]["xn"] for i in range(NCORES)], axis=0)
    an = np.concatenate([res.results[i]["an"] for i in range(NCORES)], axis=0)
    return (xn, an), res


def kernel(x, a, We, be, Wp, bp):
    (xn, an), _ = run_spmd(x, a, We, be, Wp, bp, trace=False)
    return (xn, an)


# revision 20
# speedup vs baseline: 3.0831x; 2.0859x over previous
"""DiffPool (nn_DiffPool_4715874091424) Trainium2 Bass kernel.

Math (reference is jax, B=32, C=CR=32, N=L=160, GDEP=2, ALPHA=0.05):
  A  = rownorm(a + I), A' = rownorm(a.T + I)
  mixprop folding:  embed = E0 x + E1 (M1 x) + E2 (M2 x) + 2 be
                    pool  = P0 x + P1 (M1 x) + P2 (M2 x) + 2 bp
  with M1 = A + A', M2 = A^2 + A'^2 (hop matrices), E*/P* folded 32x32
  channel-mix mats (host precompute).
  s = softmax_v(pool);  x_new[c] = s[c]^T @ embed[c];
  a_new[c] = (s[c] @ a) @ s[c].

Device pipeline per batch element b (8 cores, data-parallel over B, 4 b/core):
  1. x node-major streamed in chunks;  y12 = [M1|M2]^T.T @ x  (node matmuls)
  2. per (v,l)-segment: assemble hcat [96, seg] = [x_chan; y1_chan; y2_chan]
     (y rows via SBUF->SBUF strided DMA = the layout transpose), channel-mix
     matmul (Wcat [96,64]) + bias -> mixout -> DRAM scratch (chan-major)
  3. per c-group: reload pool/embed node-major from scratch, expP=exp(pool);
     x_new-MM with ones-column rhs yields softmax denom D as an extra output
     column; Dinv=1/D; PE-transpose expP -> s^T (Dinv-scaled on evict),
     transpose back -> s; tT = a^T s^T (const stationary); a_new = tT^T s.
"""

import sys

import numpy as np

if "/opt/trn_rl_repo" not in sys.path:
    sys.path.insert(0, "/opt/trn_rl_repo")

import concourse.bass as bass
import concourse.bacc as bacc
import concourse.mybir as mybir
import concourse.tile as tile
from concourse.bass_utils import run_bass_kernel_spmd
from concourse.masks import make_identity

F32 = mybir.dt.float32
F16 = mybir.dt.float16
AF = mybir.ActivationFunctionType

B, C, N, L = 32, 32, 160, 160
NCORES = 8
BPC = B // NCORES  # 4 batch elements per core
ALPHA, BETA = 0.05, 0.95
CL = C * L  # 5120
NSEG = 4
VQ = N // NSEG  # 20 node rows per (v,l) segment
QF = VQ * L  # 3200 free elements per segment
G = 8  # channels per phase-2 group
VT = [(0, 128), (128, 32)]  # partition tiles of the 160 node/cluster dim


def build_nc():
    nc = bacc.Bacc("TRN2", target_bir_lowering=False, debug=False, num_devices=NCORES)
    xs = nc.declare_dram_parameter("xs", [BPC, C, N, L], F16, isOutput=False)
    mt = nc.declare_dram_parameter("mt", [N, 2 * N], F16, isOutput=False)
    wcat = nc.declare_dram_parameter("wcat", [3 * C, 2 * C], F16, isOutput=False)
    b2 = nc.declare_dram_parameter("b2", [2 * C, 1], F32, isOutput=False)
    am = nc.declare_dram_parameter("am", [N, N], F16, isOutput=False)
    xn_out = nc.declare_dram_parameter("xn", [BPC, C, L, L], F32, isOutput=True)
    an_out = nc.declare_dram_parameter("an", [BPC, C, N, L], F32, isOutput=True)
    # scratch: chan-major mix output per b: [64, v, l] (embed rows 0:32, pool 32:64)
    mo = nc.dram_tensor("mo", [BPC, 2 * C, N, L], F16)
    # scratch: chan-major y1/y2 per b: [C, vstack 0:160=y1 160:320=y2, L]
    ys = nc.dram_tensor("ys", [BPC, C, 2 * N, L], F16)

    with tile.TileContext(nc) as tc:
        with (
            tc.tile_pool(name="consts", bufs=1) as pc,
            tc.tile_pool(name="work", bufs=1) as pw,
            tc.tile_pool(name="psum", bufs=1, space="PSUM") as pp,
        ):
            # ---- constants ----
            mt0 = pc.tile([128, 2 * N], F16)
            mt1 = pc.tile([32, 2 * N], F16)
            nc.sync.dma_start(mt0[:], mt[0:128, :])
            nc.sync.dma_start(mt1[:], mt[128:160, :])
            wc = pc.tile([3 * C, 2 * C], F16)
            nc.sync.dma_start(wc[:], wcat[:])
            b2c = pc.tile([2 * C, 1], F32)
            nc.sync.dma_start(b2c[:], b2[:])
            am0 = pc.tile([128, N], F16)
            am1 = pc.tile([32, N], F16)
            nc.sync.dma_start(am0[:], am[0:128, :])
            nc.sync.dma_start(am1[:], am[128:160, :])
            ident = pc.tile([128, 128], F16)
            make_identity(nc, ident[:])

            for b in range(BPC):
                _phase1(nc, pw, pp, xs, mo, ys, b, mt0, mt1, wc, b2c)
                _phase2(nc, pw, pp, mo, xn_out, an_out, b, am0, am1, ident)

    return nc


def _psA(pp, shape, dt=F32):
    return pp.tile(shape, dt, tag="psA", name="psA", bufs=4)


def _psB(pp, shape, dt=F32):
    return pp.tile(shape, dt, tag="psB", name="psB", bufs=4)


def _phase1(nc, pw, pp, xs, mo, ys, b, mt0, mt1, wc, b2c):
    """y12 node matmuls + chan-mix -> mo[b] (chan-major)."""
    xnode = xs[b].rearrange("c w l -> w c l")  # [160, 32, 160]
    MTILES = [(0, 128), (128, 128), (256, 64)]
    MTs = [mt0, mt1]

    # y-stack rows: 0:160 = y1 = M1 x, 160:320 = y2 = M2 x
    Y = [
        pw.tile([128, CL], F16, tag="y0", name="y0"),
        pw.tile([128, CL], F16, tag="y1", name="y1"),
        pw.tile([64, CL], F16, tag="y2", name="y2"),
    ]
    # x node-major streamed in groups of 8 channels ([*, 1280] tiles);
    # within a group: 4 sub-chunks of N=320 share each stationary (LDW amortized)
    for grp in range(4):
        xc0 = pw.tile([128, 1280], F16, tag="xc0", name="xc0", bufs=2)
        xc1 = pw.tile([32, 1280], F16, tag="xc1", name="xc1", bufs=2)
        nc.sync.dma_start(
            xc0[:].rearrange("p (c l) -> p c l", c=8),
            xnode[0:128, 8 * grp : 8 * grp + 8, :],
        )
        nc.sync.dma_start(
            xc1[:].rearrange("p (c l) -> p c l", c=8),
            xnode[128:160, 8 * grp : 8 * grp + 8, :],
        )
        xcs = [xc0, xc1]
        for mi, (m0, msz) in enumerate(MTILES):
            pss = [_psA(pp, [128, 320]) for _ in range(4)]
            for kt in range(2):
                for sub in range(4):
                    nc.tensor.matmul(
                        pss[sub][:msz, :],
                        MTs[kt][:, m0 : m0 + msz],
                        xcs[kt][:, sub * 320 : (sub + 1) * 320],
                        start=(kt == 0),
                        stop=(kt == 1),
                    )
            for sub in range(4):
                nc.vector.tensor_copy(
                    Y[mi][:msz, (4 * grp + sub) * 320 : (4 * grp + sub + 1) * 320],
                    pss[sub][:msz, :],
                )

    # node->chan layout transpose of y12 via DRAM roundtrip (640B runs each way)
    for mi, (m0, msz) in enumerate(MTILES):
        nc.gpsimd.dma_start(
            ys[b][:, m0 : m0 + msz, :].rearrange("c v l -> v c l"),
            Y[mi][:].rearrange("v (c l) -> v c l", c=C),
        )

    # per (v,l)-segment: hcat = [x_chan; y1_chan; y2_chan] [96, QF] -> mix
    for q in range(NSEG):
        v0 = q * VQ
        hq = pw.tile([3 * C, QF], F16, tag="hcat", name="hcat", bufs=2)
        # x rows (chan-major from DRAM)
        nc.sync.dma_start(
            hq[0:C, :].rearrange("c (v l) -> c v l", v=VQ),
            xs[b][:, v0 : v0 + VQ, :],
        )
        # y rows (chan-major from ys scratch)
        for blk, base in ((1, 0), (2, N)):  # hcat block 1 => y1, 2 => y2
            nc.sync.dma_start(
                hq[blk * C : (blk + 1) * C, :].rearrange("c (v l) -> c v l", v=VQ),
                ys[b][:, base + v0 : base + v0 + VQ, :],
            )
        # mix: out[o, pos] = sum_c' wc[c', o] * hq[c', pos], + bias
        moq = pw.tile([2 * C, QF], F16, tag="moq", name="moq", bufs=2)
        for ch in range(QF // 400):  # chunks of 400
            ps = _psB(pp, [64, 400])
            nc.tensor.matmul(
                ps[:, :], wc[:], hq[:, ch * 400 : (ch + 1) * 400],
                start=True, stop=True,
            )
            nc.vector.tensor_scalar_add(
                moq[:, ch * 400 : (ch + 1) * 400], ps[:, :], b2c[:]
            )
        nc.gpsimd.dma_start(
            mo[b][:, v0 : v0 + VQ, :],
            moq[:].rearrange("o (v l) -> o v l", v=VQ),
        )


def _phase2(nc, pw, pp, mo, xn_out, an_out, b, am0, am1, ident):
    """softmax + x_new + a_new per c-group of G."""
    ams = [am0, am1]
    for g in range(C // G):
        c0 = g * G
        # pool (rows 32:64 of mo) / embed (rows 0:32) node-major [v, (c,l)]
        pgs, egs = [], []
        for i, (v0, sz) in enumerate(VT):
            pg = pw.tile([sz, G * L], F16, tag=f"pg{i}", name=f"pg{i}", bufs=2)
            eg = pw.tile([sz, G * (L + 1)], F16, tag=f"eg{i}", name=f"eg{i}", bufs=2)
            nc.sync.dma_start(
                pg[:].rearrange("v (c l) -> v c l", c=G),
                mo[b][C + c0 : C + c0 + G, v0 : v0 + sz, :].rearrange("c v l -> v c l"),
            )
            nc.sync.dma_start(
                eg[:].rearrange("v (c l) -> v c l", c=G)[:, :, 0:L],
                mo[b][c0 : c0 + G, v0 : v0 + sz, :].rearrange("c v l -> v c l"),
            )
            nc.vector.memset(
                eg[:].rearrange("v (c l) -> v c l", c=G)[:, :, L : L + 1], 1.0
            )
            pgs.append(pg)
            egs.append(eg)
        # expP = exp(pool)
        xps = [
            pw.tile([sz, G * L], F16, tag=f"xp{i}", name=f"xp{i}", bufs=2)
            for i, (_, sz) in enumerate(VT)
        ]
        for i in range(2):
            nc.scalar.activation(xps[i][:], pgs[i][:], AF.Exp)
        dvs = [
            pw.tile([sz, G], F32, tag=f"dv{i}", name=f"dv{i}")
            for i, (_, sz) in enumerate(VT)
        ]
        slg = [
            pw.tile([sz, G * N], F16, tag=f"sl{i}", name=f"sl{i}", bufs=2)
            for i, (_, sz) in enumerate(VT)
        ]
        sng = [
            pw.tile([sz, G * L], F16, tag=f"sn{i}", name=f"sn{i}", bufs=2)
            for i, (_, sz) in enumerate(VT)
        ]
        xgs = [
            pw.tile([sz, G * L], F32, tag=f"xg{i}", name=f"xg{i}", bufs=2)
            for i, (_, sz) in enumerate(VT)
        ]

        for ci in range(G):
            # ---- x_new: raw = expP[c]^T @ [e[c] | 1];  D = last col ----
            for mi, (m0, msz) in enumerate(VT):  # l1 tiles
                ps = _psA(pp, [128, L + 1])
                for kt, (k0, ksz) in enumerate(VT):  # v tiles
                    nc.tensor.matmul(
                        ps[:msz, :],
                        xps[kt][:, ci * L + m0 : ci * L + m0 + msz],
                        egs[kt][:, ci * (L + 1) : (ci + 1) * (L + 1)],
                        start=(kt == 0),
                        stop=(kt == 1),
                    )
                nc.vector.reciprocal(dvs[mi][:msz, ci : ci + 1], ps[:msz, L : L + 1])
                nc.scalar.activation(
                    xgs[mi][:msz, ci * L : (ci + 1) * L],
                    ps[:msz, 0:L],
                    AF.Copy,
                    scale=dvs[mi][:msz, ci : ci + 1],
                )
            # ---- T1: s^T[c] = transpose(expP[c]) * Dinv  (l-major) ----
            for kt, (k0, ksz) in enumerate(VT):  # source v tile
                for lt, (l0, lsz) in enumerate(VT):  # source l cols
                    ps = _psA(pp, [128, 128], F16)
                    nc.tensor.transpose(
                        ps[:lsz, :ksz],
                        xps[kt][:, ci * L + l0 : ci * L + l0 + lsz],
                        ident[:ksz, :ksz],
                    )
                    nc.scalar.activation(
                        slg[lt][:lsz, ci * N + k0 : ci * N + k0 + ksz],
                        ps[:lsz, :ksz],
                        AF.Copy,
                        scale=dvs[lt][:lsz, ci : ci + 1],
                    )
            # ---- T2: s[c] = transpose(s^T[c])  (node-major) ----
            for kt, (k0, ksz) in enumerate(VT):  # source l tile
                for vt, (v0, vsz) in enumerate(VT):  # source v cols
                    ps = _psB(pp, [128, 128], F16)
                    nc.tensor.transpose(
                        ps[:vsz, :ksz],
                        slg[kt][:, ci * N + v0 : ci * N + v0 + vsz],
                        ident[:ksz, :ksz],
                    )
                    nc.vector.tensor_copy(
                        sng[vt][:vsz, ci * L + k0 : ci * L + k0 + ksz],
                        ps[:vsz, :ksz],
                    )
        # ---- tT = a^T s^T : tT[j, (c,v)] = sum_k a[k,j] s_l[k, (c,v)] ----
        ttg = [
            pw.tile([sz, G * N], F16, tag=f"tt{i}", name=f"tt{i}", bufs=2)
            for i, (_, sz) in enumerate(VT)
        ]
        NCH = G * N // 320  # chunks of 320
        for mi, (m0, msz) in enumerate(VT):  # j tiles
            for ch in range(NCH):
                ps = _psB(pp, [128, 320])
                for kt in range(2):
                    nc.tensor.matmul(
                        ps[:msz, :],
                        ams[kt][:, m0 : m0 + msz],
                        slg[kt][:, ch * 320 : (ch + 1) * 320],
                        start=(kt == 0),
                        stop=(kt == 1),
                    )
                nc.vector.tensor_copy(
                    ttg[mi][:msz, ch * 320 : (ch + 1) * 320], ps[:msz, :]
                )
        # ---- a_new[c] = tT[c]^T @ s[c] ----
        ang = [
            pw.tile([sz, G * L], F32, tag=f"ag{i}", name=f"ag{i}", bufs=2)
            for i, (_, sz) in enumerate(VT)
        ]
        for ci in range(G):
            for mi, (m0, msz) in enumerate(VT):  # v tiles (output partition)
                ps = _psB(pp, [128, L])
                for kt, (k0, ksz) in enumerate(VT):  # j tiles
                    nc.tensor.matmul(
                        ps[:msz, :],
                        ttg[kt][:, ci * N + m0 : ci * N + m0 + msz],
                        sng[kt][:, ci * L : (ci + 1) * L],
                        start=(kt == 0),
                        stop=(kt == 1),
                    )
                nc.vector.tensor_copy(ang[mi][:msz, ci * L : (ci + 1) * L], ps[:msz, :])
        # ---- outputs ----
        for i, (v0, sz) in enumerate(VT):
            nc.gpsimd.dma_start(
                xn_out[b][c0 : c0 + G, v0 : v0 + sz, :].rearrange("c p q -> p c q"),
                xgs[i][:sz].rearrange("p (c q) -> p c q", c=G),
            )
            nc.gpsimd.dma_start(
                an_out[b][c0 : c0 + G, v0 : v0 + sz, :].rearrange("c p q -> p c q"),
                ang[i][:sz].rearrange("p (c q) -> p c q", c=G),
            )


def _host_prep(x, a, We, be, Wp, bp):
    a = np.asarray(a, np.float64)
    I = np.eye(N, dtype=np.float64)
    A1 = (a + I) / (a + I).sum(1, keepdims=True)
    A2 = (a.T + I) / (a.T + I).sum(1, keepdims=True)
    M1 = A1 + A2
    M2 = A1 @ A1 + A2 @ A2
    MT = np.concatenate([M1.T, M2.T], axis=1).astype(np.float16)  # [N, 2N]

    def fold(W):
        W = np.asarray(W, np.float64)
        W0, W1, W2 = W[:, :C], W[:, C : 2 * C], W[:, 2 * C :]
        F0 = 2.0 * (W0 + ALPHA * W1 + ALPHA * W2)
        F1 = BETA * W1 + ALPHA * BETA * W2
        F2 = BETA * BETA * W2
        return F0, F1, F2

    E0, E1, E2 = fold(We)
    P0, P1, P2 = fold(Wp)
    # lhsT[c', o]: rows = [x-block; y1-block; y2-block], cols = [e outs | pool outs]
    Wcat = np.block([[E0.T, P0.T], [E1.T, P1.T], [E2.T, P2.T]]).astype(np.float16)
    b2 = np.concatenate([2.0 * np.asarray(be), 2.0 * np.asarray(bp)]).astype(
        np.float32
    )[:, None]
    return MT, Wcat, b2, np.asarray(a, np.float16)


def _install_ntff_shim():
    """Provide antenv.axon_hooks (missing in this image) so
    run_bass_kernel_spmd(trace=True) can drive NTFF profiling via the
    axon PJRT .so. No-op if anything is unavailable."""
    import contextlib
    import ctypes
    import types

    try:
        import antenv  # noqa: F401

        try:
            from antenv.axon_hooks import get_axon_ntff_profile_hook  # noqa: F401

            return
        except ImportError:
            pass
        lib = ctypes.CDLL("/opt/axon/libaxon_pjrt.so")
        if not hasattr(lib, "axon_start_nrt_profile"):
            return
        lib.axon_start_nrt_profile.argtypes = [
            ctypes.POINTER(ctypes.c_int64),
            ctypes.c_size_t,
        ]
        lib.axon_start_nrt_profile.restype = ctypes.c_int64
        lib.axon_stop_nrt_profile.argtypes = [ctypes.c_char_p]
        lib.axon_stop_nrt_profile.restype = ctypes.c_int64

        @contextlib.contextmanager
        def _hook(output_dir, device_ids):
            import jax

            jax.devices()
            if device_ids:
                ids = (ctypes.c_int64 * len(device_ids))(*device_ids)
                rc = lib.axon_start_nrt_profile(ids, len(device_ids))
            else:
                rc = lib.axon_start_nrt_profile(None, 0)
            if rc != 0:
                raise RuntimeError(f"axon_start_nrt_profile rc={rc}")
            try:
                yield
            finally:
                n = lib.axon_stop_nrt_profile(str(output_dir).encode())
                print(f"ntff profile: {n} file(s) -> {output_dir}", file=sys.stderr)

        holder = {"h": _hook}
        mod = types.ModuleType("antenv.axon_hooks")
        mod.get_axon_ntff_profile_hook = lambda: holder["h"]
        mod.set_axon_ntff_profile_hook = lambda h: holder.__setitem__("h", h)
        sys.modules["antenv.axon_hooks"] = mod
        antenv.axon_hooks = mod
    except Exception as e:  # pragma: no cover
        print(f"ntff shim unavailable: {e}", file=sys.stderr)


_NC_CACHE = {}


def _get_nc():
    if "nc" not in _NC_CACHE:
        nc = build_nc()
        nc.compile()  # bacc lowering: wait-splitting, register allocation, ...
        _NC_CACHE["nc"] = nc
    return _NC_CACHE["nc"]


def run_spmd(x, a, We, be, Wp, bp, trace=False):
    if trace:
        _install_ntff_shim()
    x16 = np.ascontiguousarray(np.asarray(x, np.float16))
    MT, Wcat, b2, a32 = _host_prep(x, a, We, be, Wp, bp)
    nc = _get_nc()
    in_maps = [
        {
            "xs": x16[i * BPC : (i + 1) * BPC],
            "mt": MT,
            "wcat": Wcat,
            "b2": b2,
            "am": a32,
        }
        for i in range(NCORES)
    ]
    res = run_bass_kernel_spmd(nc, in_maps, list(range(NCORES)), trace=trace)
    xn = np.concatenate([res.results[i]["xn"] for i in range(NCORES)], axis=0)
    an = np.concatenate([res.results[i]["an"] for i in range(NCORES)], axis=0)
    return (xn, an), res


def kernel(x, a, We, be, Wp, bp):
    (xn, an), _ = run_spmd(x, a, We, be, Wp, bp, trace=False)
    return (xn, an)


# revision 23
# speedup vs baseline: 3.3575x; 1.0890x over previous
"""DiffPool (nn_DiffPool_4715874091424) Trainium2 Bass kernel.

Math (reference is jax, B=32, C=CR=32, N=L=160, GDEP=2, ALPHA=0.05):
  A  = rownorm(a + I), A' = rownorm(a.T + I)
  mixprop folding:  embed = E0 x + E1 (M1 x) + E2 (M2 x) + 2 be
                    pool  = P0 x + P1 (M1 x) + P2 (M2 x) + 2 bp
  with M1 = A + A', M2 = A^2 + A'^2 (hop matrices), E*/P* folded 32x32
  channel-mix mats (host precompute).
  s = softmax_v(pool);  x_new[c] = s[c]^T @ embed[c];
  a_new[c] = (s[c] @ a) @ s[c].

Device pipeline per batch element b (8 cores, data-parallel over B, 4 b/core):
  1. x node-major streamed in chunks;  y12 = [M1|M2]^T.T @ x  (node matmuls)
  2. per (v,l)-segment: assemble hcat [96, seg] = [x_chan; y1_chan; y2_chan]
     (y rows via SBUF->SBUF strided DMA = the layout transpose), channel-mix
     matmul (Wcat [96,64]) + bias -> mixout -> DRAM scratch (chan-major)
  3. per c-group: reload pool/embed node-major from scratch, expP=exp(pool);
     x_new-MM with ones-column rhs yields softmax denom D as an extra output
     column; Dinv=1/D; PE-transpose expP -> s^T (Dinv-scaled on evict),
     transpose back -> s; tT = a^T s^T (const stationary); a_new = tT^T s.
"""

import sys

import numpy as np

if "/opt/trn_rl_repo" not in sys.path:
    sys.path.insert(0, "/opt/trn_rl_repo")

import concourse.bass as bass
import concourse.bacc as bacc
import concourse.mybir as mybir
import concourse.tile as tile
from concourse.bass_utils import run_bass_kernel_spmd
from concourse.masks import make_identity

F32 = mybir.dt.float32
F16 = mybir.dt.float16
AF = mybir.ActivationFunctionType

B, C, N, L = 32, 32, 160, 160
NCORES = 8
BPC = B // NCORES  # 4 batch elements per core
ALPHA, BETA = 0.05, 0.95
CL = C * L  # 5120
NSEG = 4
VQ = N // NSEG  # 20 node rows per (v,l) segment
QF = VQ * L  # 3200 free elements per segment
G = 8  # channels per phase-2 group
VT = [(0, 128), (128, 32)]  # partition tiles of the 160 node/cluster dim


def build_nc():
    nc = bacc.Bacc("TRN2", target_bir_lowering=False, debug=False, num_devices=NCORES)
    xs = nc.declare_dram_parameter("xs", [BPC, C, N, L], F16, isOutput=False)
    mt = nc.declare_dram_parameter("mt", [N, 2 * N], F16, isOutput=False)
    wcat = nc.declare_dram_parameter("wcat", [3 * C, 2 * C], F16, isOutput=False)
    b2 = nc.declare_dram_parameter("b2", [2 * C, 1], F32, isOutput=False)
    am = nc.declare_dram_parameter("am", [N, N], F16, isOutput=False)
    xn_out = nc.declare_dram_parameter("xn", [BPC, C, L, L], F32, isOutput=True)
    an_out = nc.declare_dram_parameter("an", [BPC, C, N, L], F32, isOutput=True)
    # scratch: chan-major mix output per b: [64, v, l] (embed rows 0:32, pool 32:64)
    mo = nc.dram_tensor("mo", [BPC, 2 * C, N, L], F16)
    # scratch: chan-major y1/y2 per b: [C, vstack 0:160=y1 160:320=y2, L]
    ys = nc.dram_tensor("ys", [BPC, C, 2 * N, L], F16)

    with tile.TileContext(nc) as tc:
        with (
            tc.tile_pool(name="consts", bufs=1) as pc,
            tc.tile_pool(name="work", bufs=1) as pw,
            tc.tile_pool(name="psum", bufs=1, space="PSUM") as pp,
        ):
            # ---- constants ----
            mt0 = pc.tile([128, 2 * N], F16)
            mt1 = pc.tile([32, 2 * N], F16)
            nc.sync.dma_start(mt0[:], mt[0:128, :])
            nc.sync.dma_start(mt1[:], mt[128:160, :])
            wc = pc.tile([3 * C, 2 * C], F16)
            nc.sync.dma_start(wc[:], wcat[:])
            b2c = pc.tile([2 * C, 1], F32)
            nc.sync.dma_start(b2c[:], b2[:])
            am0 = pc.tile([128, N], F16)
            am1 = pc.tile([32, N], F16)
            nc.sync.dma_start(am0[:], am[0:128, :])
            nc.sync.dma_start(am1[:], am[128:160, :])
            ident = pc.tile([128, 128], F16)
            make_identity(nc, ident[:])

            for b in range(BPC):
                _phase1(nc, pw, pp, xs, mo, ys, b, mt0, mt1, wc, b2c)
                _phase2(nc, pw, pp, mo, xn_out, an_out, b, am0, am1, ident)

    return nc


def _psA(pp, shape, dt=F32):
    return pp.tile(shape, dt, tag="psA", name="psA", bufs=4)


def _psB(pp, shape, dt=F32):
    return pp.tile(shape, dt, tag="psB", name="psB", bufs=4)


def _phase1(nc, pw, pp, xs, mo, ys, b, mt0, mt1, wc, b2c):
    """y12 node matmuls + chan-mix -> mo[b] (chan-major)."""
    xnode = xs[b].rearrange("c w l -> w c l")  # [160, 32, 160]
    MTILES = [(0, 128), (128, 128), (256, 64)]
    MTs = [mt0, mt1]

    # y-stack rows: 0:160 = y1 = M1 x, 160:320 = y2 = M2 x
    Y = [
        pw.tile([128, CL], F16, tag="y0", name="y0"),
        pw.tile([128, CL], F16, tag="y1", name="y1"),
        pw.tile([64, CL], F16, tag="y2", name="y2"),
    ]
    # x node-major: one flat [*, 5120] tile per K-tile (single DMA each);
    # rhs chunks of N=512 slice the flat free dim, LDW amortized over 4-chunk
    # groups per stationary
    xc0 = pw.tile([128, CL], F16, tag="xc0", name="xc0", bufs=1)
    xc1 = pw.tile([32, CL], F16, tag="xc1", name="xc1", bufs=1)
    nc.sync.dma_start(xc0[:].rearrange("p (c l) -> p c l", c=C), xnode[0:128])
    nc.sync.dma_start(xc1[:].rearrange("p (c l) -> p c l", c=C), xnode[128:160])
    xcs = [xc0, xc1]
    for mi, (m0, msz) in enumerate(MTILES):
        for sg in range(0, 10, 4):  # chunk groups of <=4 (N=512 each)
            subs = range(sg, min(sg + 4, 10))
            pss = {sub: _psA(pp, [128, 512]) for sub in subs}
            for kt in range(2):
                for sub in subs:
                    nc.tensor.matmul(
                        pss[sub][:msz, :],
                        MTs[kt][:, m0 : m0 + msz],
                        xcs[kt][:, sub * 512 : (sub + 1) * 512],
                        start=(kt == 0),
                        stop=(kt == 1),
                    )
            for sub in subs:
                nc.vector.tensor_copy(
                    Y[mi][:msz, sub * 512 : (sub + 1) * 512], pss[sub][:msz, :]
                )

    # node->chan layout transpose of y12 via DRAM roundtrip (640B runs each way)
    for mi, (m0, msz) in enumerate(MTILES):
        nc.gpsimd.dma_start(
            ys[b][:, m0 : m0 + msz, :].rearrange("c v l -> v c l"),
            Y[mi][:].rearrange("v (c l) -> v c l", c=C),
        )

    # per (v,l)-segment: hcat = [x_chan; y1_chan; y2_chan] [96, QF] -> mix
    for q in range(NSEG):
        v0 = q * VQ
        hq = pw.tile([3 * C, QF], F16, tag="hcat", name="hcat", bufs=2)
        # x rows (chan-major from DRAM)
        nc.sync.dma_start(
            hq[0:C, :].rearrange("c (v l) -> c v l", v=VQ),
            xs[b][:, v0 : v0 + VQ, :],
        )
        # y rows (chan-major from ys scratch)
        for blk, base in ((1, 0), (2, N)):  # hcat block 1 => y1, 2 => y2
            nc.sync.dma_start(
                hq[blk * C : (blk + 1) * C, :].rearrange("c (v l) -> c v l", v=VQ),
                ys[b][:, base + v0 : base + v0 + VQ, :],
            )
        # mix: out[o, pos] = sum_c' wc[c', o] * hq[c', pos], + bias
        moq = pw.tile([2 * C, QF], F16, tag="moq", name="moq", bufs=2)
        for off in range(0, QF, 512):  # chunks of 512 over the flat free dim
            nn = min(512, QF - off)
            ps = _psB(pp, [64, 512])
            nc.tensor.matmul(
                ps[:, :nn], wc[:], hq[:, off : off + nn], start=True, stop=True
            )
            nc.vector.tensor_scalar_add(
                moq[:, off : off + nn], ps[:, :nn], b2c[:]
            )
        nc.gpsimd.dma_start(
            mo[b][:, v0 : v0 + VQ, :],
            moq[:].rearrange("o (v l) -> o v l", v=VQ),
        )


def _phase2(nc, pw, pp, mo, xn_out, an_out, b, am0, am1, ident):
    """softmax + x_new + a_new per c-group of G."""
    ams = [am0, am1]
    for g in range(C // G):
        c0 = g * G
        # pool (rows 32:64 of mo) / embed (rows 0:32) node-major [v, (c,l)]
        pgs, egs = [], []
        for i, (v0, sz) in enumerate(VT):
            pg = pw.tile([sz, G * L], F16, tag=f"pg{i}", name=f"pg{i}", bufs=2)
            eg = pw.tile([sz, G * (L + 1)], F16, tag=f"eg{i}", name=f"eg{i}", bufs=2)
            nc.sync.dma_start(
                pg[:].rearrange("v (c l) -> v c l", c=G),
                mo[b][C + c0 : C + c0 + G, v0 : v0 + sz, :].rearrange("c v l -> v c l"),
            )
            nc.sync.dma_start(
                eg[:].rearrange("v (c l) -> v c l", c=G)[:, :, 0:L],
                mo[b][c0 : c0 + G, v0 : v0 + sz, :].rearrange("c v l -> v c l"),
            )
            nc.vector.memset(
                eg[:].rearrange("v (c l) -> v c l", c=G)[:, :, L : L + 1], 1.0
            )
            pgs.append(pg)
            egs.append(eg)
        # expP = exp(pool)
        xps = [
            pw.tile([sz, G * L], F16, tag=f"xp{i}", name=f"xp{i}", bufs=2)
            for i, (_, sz) in enumerate(VT)
        ]
        for i in range(2):
            nc.scalar.activation(xps[i][:], pgs[i][:], AF.Exp)
        dvs = [
            pw.tile([sz, G], F32, tag=f"dv{i}", name=f"dv{i}")
            for i, (_, sz) in enumerate(VT)
        ]
        slg = [
            pw.tile([sz, G * N], F16, tag=f"sl{i}", name=f"sl{i}", bufs=2)
            for i, (_, sz) in enumerate(VT)
        ]
        sng = [
            pw.tile([sz, G * L], F16, tag=f"sn{i}", name=f"sn{i}", bufs=2)
            for i, (_, sz) in enumerate(VT)
        ]
        xgs = [
            pw.tile([sz, G * L], F32, tag=f"xg{i}", name=f"xg{i}", bufs=2)
            for i, (_, sz) in enumerate(VT)
        ]

        for ci in range(G):
            # ---- x_new: raw = expP[c]^T @ [e[c] | 1];  D = last col ----
            for mi, (m0, msz) in enumerate(VT):  # l1 tiles
                ps = _psA(pp, [128, L + 1])
                for kt, (k0, ksz) in enumerate(VT):  # v tiles
                    nc.tensor.matmul(
                        ps[:msz, :],
                        xps[kt][:, ci * L + m0 : ci * L + m0 + msz],
                        egs[kt][:, ci * (L + 1) : (ci + 1) * (L + 1)],
                        start=(kt == 0),
                        stop=(kt == 1),
                    )
                nc.vector.reciprocal(dvs[mi][:msz, ci : ci + 1], ps[:msz, L : L + 1])
                nc.scalar.activation(
                    xgs[mi][:msz, ci * L : (ci + 1) * L],
                    ps[:msz, 0:L],
                    AF.Copy,
                    scale=dvs[mi][:msz, ci : ci + 1],
                )
            # ---- T1: s^T[c] = transpose(expP[c]) * Dinv  (l-major) ----
            for kt, (k0, ksz) in enumerate(VT):  # source v tile
                for lt, (l0, lsz) in enumerate(VT):  # source l cols
                    ps = _psA(pp, [128, 128], F16)
                    nc.tensor.transpose(
                        ps[:lsz, :ksz],
                        xps[kt][:, ci * L + l0 : ci * L + l0 + lsz],
                        ident[:ksz, :ksz],
                    )
                    nc.scalar.activation(
                        slg[lt][:lsz, ci * N + k0 : ci * N + k0 + ksz],
                        ps[:lsz, :ksz],
                        AF.Copy,
                        scale=dvs[lt][:lsz, ci : ci + 1],
                    )
            # ---- T2: s[c] = transpose(s^T[c])  (node-major) ----
            for kt, (k0, ksz) in enumerate(VT):  # source l tile
                for vt, (v0, vsz) in enumerate(VT):  # source v cols
                    ps = _psB(pp, [128, 128], F16)
                    nc.tensor.transpose(
                        ps[:vsz, :ksz],
                        slg[kt][:, ci * N + v0 : ci * N + v0 + vsz],
                        ident[:ksz, :ksz],
                    )
                    nc.vector.tensor_copy(
                        sng[vt][:vsz, ci * L + k0 : ci * L + k0 + ksz],
                        ps[:vsz, :ksz],
                    )
        # ---- tT = a^T s^T : tT[j, (c,v)] = sum_k a[k,j] s_l[k, (c,v)] ----
        ttg = [
            pw.tile([sz, G * N], F16, tag=f"tt{i}", name=f"tt{i}", bufs=2)
            for i, (_, sz) in enumerate(VT)
        ]
        NCH = G * N // 320  # chunks of 320
        for mi, (m0, msz) in enumerate(VT):  # j tiles
            for ch in range(NCH):
                ps = _psB(pp, [128, 320])
                for kt in range(2):
                    nc.tensor.matmul(
                        ps[:msz, :],
                        ams[kt][:, m0 : m0 + msz],
                        slg[kt][:, ch * 320 : (ch + 1) * 320],
                        start=(kt == 0),
                        stop=(kt == 1),
                    )
                nc.vector.tensor_copy(
                    ttg[mi][:msz, ch * 320 : (ch + 1) * 320], ps[:msz, :]
                )
        # ---- a_new[c] = tT[c]^T @ s[c] ----
        ang = [
            pw.tile([sz, G * L], F32, tag=f"ag{i}", name=f"ag{i}", bufs=2)
            for i, (_, sz) in enumerate(VT)
        ]
        for ci in range(G):
            for mi, (m0, msz) in enumerate(VT):  # v tiles (output partition)
                ps = _psB(pp, [128, L])
                for kt, (k0, ksz) in enumerate(VT):  # j tiles
                    nc.tensor.matmul(
                        ps[:msz, :],
                        ttg[kt][:, ci * N + m0 : ci * N + m0 + msz],
                        sng[kt][:, ci * L : (ci + 1) * L],
                        start=(kt == 0),
                        stop=(kt == 1),
                    )
                nc.vector.tensor_copy(ang[mi][:msz, ci * L : (ci + 1) * L], ps[:msz, :])
        # ---- outputs ----
        for i, (v0, sz) in enumerate(VT):
            nc.gpsimd.dma_start(
                xn_out[b][c0 : c0 + G, v0 : v0 + sz, :].rearrange("c p q -> p c q"),
                xgs[i][:sz].rearrange("p (c q) -> p c q", c=G),
            )
            nc.gpsimd.dma_start(
                an_out[b][c0 : c0 + G, v0 : v0 + sz, :].rearrange("c p q -> p c q"),
                ang[i][:sz].rearrange("p (c q) -> p c q", c=G),
            )


def _host_prep(x, a, We, be, Wp, bp):
    a = np.asarray(a, np.float64)
    I = np.eye(N, dtype=np.float64)
    A1 = (a + I) / (a + I).sum(1, keepdims=True)
    A2 = (a.T + I) / (a.T + I).sum(1, keepdims=True)
    M1 = A1 + A2
    M2 = A1 @ A1 + A2 @ A2
    MT = np.concatenate([M1.T, M2.T], axis=1).astype(np.float16)  # [N, 2N]

    def fold(W):
        W = np.asarray(W, np.float64)
        W0, W1, W2 = W[:, :C], W[:, C : 2 * C], W[:, 2 * C :]
        F0 = 2.0 * (W0 + ALPHA * W1 + ALPHA * W2)
        F1 = BETA * W1 + ALPHA * BETA * W2
        F2 = BETA * BETA * W2
        return F0, F1, F2

    E0, E1, E2 = fold(We)
    P0, P1, P2 = fold(Wp)
    # lhsT[c', o]: rows = [x-block; y1-block; y2-block], cols = [e outs | pool outs]
    Wcat = np.block([[E0.T, P0.T], [E1.T, P1.T], [E2.T, P2.T]]).astype(np.float16)
    b2 = np.concatenate([2.0 * np.asarray(be), 2.0 * np.asarray(bp)]).astype(
        np.float32
    )[:, None]
    return MT, Wcat, b2, np.asarray(a, np.float16)


def _install_ntff_shim():
    """Provide antenv.axon_hooks (missing in this image) so
    run_bass_kernel_spmd(trace=True) can drive NTFF profiling via the
    axon PJRT .so. No-op if anything is unavailable."""
    import contextlib
    import ctypes
    import types

    try:
        import antenv  # noqa: F401

        try:
            from antenv.axon_hooks import get_axon_ntff_profile_hook  # noqa: F401

            return
        except ImportError:
            pass
        lib = ctypes.CDLL("/opt/axon/libaxon_pjrt.so")
        if not hasattr(lib, "axon_start_nrt_profile"):
            return
        lib.axon_start_nrt_profile.argtypes = [
            ctypes.POINTER(ctypes.c_int64),
            ctypes.c_size_t,
        ]
        lib.axon_start_nrt_profile.restype = ctypes.c_int64
        lib.axon_stop_nrt_profile.argtypes = [ctypes.c_char_p]
        lib.axon_stop_nrt_profile.restype = ctypes.c_int64

        @contextlib.contextmanager
        def _hook(output_dir, device_ids):
            import jax

            jax.devices()
            if device_ids:
                ids = (ctypes.c_int64 * len(device_ids))(*device_ids)
                rc = lib.axon_start_nrt_profile(ids, len(device_ids))
            else:
                rc = lib.axon_start_nrt_profile(None, 0)
            if rc != 0:
                raise RuntimeError(f"axon_start_nrt_profile rc={rc}")
            try:
                yield
            finally:
                n = lib.axon_stop_nrt_profile(str(output_dir).encode())
                print(f"ntff profile: {n} file(s) -> {output_dir}", file=sys.stderr)

        holder = {"h": _hook}
        mod = types.ModuleType("antenv.axon_hooks")
        mod.get_axon_ntff_profile_hook = lambda: holder["h"]
        mod.set_axon_ntff_profile_hook = lambda h: holder.__setitem__("h", h)
        sys.modules["antenv.axon_hooks"] = mod
        antenv.axon_hooks = mod
    except Exception as e:  # pragma: no cover
        print(f"ntff shim unavailable: {e}", file=sys.stderr)


_NC_CACHE = {}


def _get_nc():
    if "nc" not in _NC_CACHE:
        nc = build_nc()
        nc.compile()  # bacc lowering: wait-splitting, register allocation, ...
        _NC_CACHE["nc"] = nc
    return _NC_CACHE["nc"]


def run_spmd(x, a, We, be, Wp, bp, trace=False):
    if trace:
        _install_ntff_shim()
    x16 = np.ascontiguousarray(np.asarray(x, np.float16))
    MT, Wcat, b2, a32 = _host_prep(x, a, We, be, Wp, bp)
    nc = _get_nc()
    in_maps = [
        {
            "xs": x16[i * BPC : (i + 1) * BPC],
            "mt": MT,
            "wcat": Wcat,
            "b2": b2,
            "am": a32,
        }
        for i in range(NCORES)
    ]
    res = run_bass_kernel_spmd(nc, in_maps, list(range(NCORES)), trace=trace)
    xn = np.concatenate([res.results[i]["xn"] for i in range(NCORES)], axis=0)
    an = np.concatenate([res.results[i]["an"] for i in range(NCORES)], axis=0)
    return (xn, an), res


def kernel(x, a, We, be, Wp, bp):
    (xn, an), _ = run_spmd(x, a, We, be, Wp, bp, trace=False)
    return (xn, an)
